# revision 20
# baseline (speedup 1.0000x reference)
"""Trainium2 Bass kernel for a dense pre-LN decoder layer (MHA + FFN).

Sharding (8 NeuronCores, one chip):
  - Attention: tensor-parallel over heads (16 heads -> 2 per core).
  - FFN: tensor-parallel over the 4*d_model hidden dim (8192 -> 1024 per core).
  - LayerNorms: sequence-parallel. Core c owns token blocks
    {u*1024 + c*128 : u in 0..3} so that each chunk-u ReduceScatter /
    AllGather maps to the contiguous global token range [u*1024,(u+1)*1024).
    All collectives are chunked (4x) and pipeline behind compute.
  - Attention-output partials: chunked ReduceScatter; FFN partials are
    summed on the host (the gather/unshard step).

All matmuls run in bf16 with fp32 PSUM accumulation. Scores are computed
transposed (scT[k,q] = kT.T @ qT) with host-pre-transposed alibi(+causal
mask), so exp() directly yields E^T = the AV-matmul rhs - no on-device
transposes in attention. Softmax denominators via a ones-row matmul on PE.
LN gains/biases and 1/sqrt(head_dim) are folded into weights on the host.
"""

import math
import sys

import numpy as np

sys.path.insert(0, "/opt/trn_rl_repo")

import concourse.bass as bass  # noqa: E402
import concourse.tile as tile  # noqa: E402
from concourse import bacc, mybir  # noqa: E402
from concourse.bass_utils import run_bass_kernel_spmd  # noqa: E402

try:
    from ml_dtypes import bfloat16 as np_bf16
except ImportError:  # pragma: no cover
    import jax.numpy as jnp

    np_bf16 = jnp.bfloat16

# ---------------------------------------------------------------- constants
NCORES = 8
D = 2048          # d_model
S = 2048          # sequence length
B = 2             # batch
NTOK = B * S      # 4096 global tokens
HD = 128          # head dim
NH = 16           # total heads
HPC = NH // NCORES      # heads per core = 2
DFF = 4 * D             # 8192
DFFC = DFF // NCORES    # ffn hidden per core = 1024
TOKC = NTOK // NCORES   # tokens per core (sequence-parallel) = 512
LN_EPS = 1e-5
NEG = -1.0e30

P = 128           # SBUF partitions
SPAN = 512        # token span for matmul rhs
NQB = S // P                # 16 q blocks per batch element
KCH = D // P                # 16 contraction chunks of 128 over d_model
FSL = DFFC // P             # 8 ffn slices of 128
NSP = S // SPAN             # 4 q spans per sequence
NU = 4                      # collective chunks; chunk u = tokens [u*1024, ...)
UTOK = NTOK // NU           # 1024 tokens per chunk
F32 = mybir.dt.float32
BF16 = mybir.dt.bfloat16

_CACHE = {}
LAST_RESULT = None


# ---------------------------------------------------------------- program
def build_program():
    nc = bacc.Bacc(
        "TRN2", target_bir_lowering=False, debug=False, num_devices=NCORES
    )

    # -------- per-core I/O (same shapes on every core; data differs)
    xc = nc.dram_tensor("xc", [TOKC, D], F32, kind="ExternalInput").ap()
    alibi = nc.dram_tensor("alibi", [HPC, S, S], F32, kind="ExternalInput").ap()
    wq = nc.dram_tensor("wq", [D, HPC * HD], BF16, kind="ExternalInput").ap()
    wk = nc.dram_tensor("wk", [D, HPC * HD], BF16, kind="ExternalInput").ap()
    wv = nc.dram_tensor("wv", [D, HPC * HD], BF16, kind="ExternalInput").ap()
    bqkv = nc.dram_tensor("bqkv", [3 * HPC * HD], F32, kind="ExternalInput").ap()
    wo = nc.dram_tensor("wo", [HPC * HD, D], BF16, kind="ExternalInput").ap()
    w1 = nc.dram_tensor("w1", [D, DFFC], BF16, kind="ExternalInput").ap()
    b1 = nc.dram_tensor("b1", [DFFC], F32, kind="ExternalInput").ap()
    w2 = nc.dram_tensor("w2", [DFFC, D], BF16, kind="ExternalInput").ap()

    ffn_part = nc.dram_tensor(
        "ffn_part", [NTOK, D], BF16, kind="ExternalOutput"
    ).ap()
    h_part = nc.dram_tensor("h_part", [TOKC, D], F32, kind="ExternalOutput").ap()

    groups = [list(range(NCORES))]

    with tile.TileContext(nc) as tc:
        _build(tc, nc, xc, alibi, wq, wk, wv, bqkv, wo, w1, b1, w2,
               ffn_part, h_part, groups)

    nc.compile()
    return nc


def _ln_tile(nc, pool, eps_t, src_ap, dst_ap, tag, rs_ap=None, h_ap=None):
    """One 128-row LayerNorm: read f32 rows from src_ap, write bf16
    normalized rows to dst_ap. With rs_ap: add the bf16 residual first and
    store the f32 sum to h_ap."""
    x_t = pool.tile([P, D], F32, tag="ln_x", name=f"ln_x_{tag}")
    nc.gpsimd.dma_start(out=x_t[:], in_=src_ap)
    if rs_ap is not None:
        rs_t = pool.tile([P, D], BF16, tag="ln_rs", name=f"ln_rs_{tag}")
        nc.gpsimd.dma_start(out=rs_t[:], in_=rs_ap)
        nc.vector.tensor_add(out=x_t[:], in0=x_t[:], in1=rs_t[:])
        nc.gpsimd.dma_start(out=h_ap, in_=x_t[:])
    x_view = x_t[:].rearrange("p (c f) -> p c f", f=512)
    st = pool.tile([P, 4, 6], F32, tag="ln_st", name=f"ln_st_{tag}")
    for c in range(4):
        nc.vector.bn_stats(out=st[:, c, :], in_=x_view[:, c, :])
    mv = pool.tile([P, 2], F32, tag="ln_mv", name=f"ln_mv_{tag}")
    nc.vector.bn_aggr(out=mv[:], in_=st[:])
    inv = pool.tile([P, 1], F32, tag="ln_inv", name=f"ln_inv_{tag}")
    nc.scalar.activation(out=inv[:], in_=mv[:, 1:2],
                         func=mybir.ActivationFunctionType.Sqrt,
                         bias=eps_t[:], scale=1.0)
    nc.vector.reciprocal(out=inv[:], in_=inv[:])
    xh = pool.tile([P, D], BF16, tag="ln_xh", name=f"ln_xh_{tag}")
    nc.vector.tensor_scalar(
        out=xh[:], in0=x_t[:], scalar1=mv[:, 0:1], scalar2=inv[:],
        op0=mybir.AluOpType.subtract, op1=mybir.AluOpType.mult)
    nc.gpsimd.dma_start(out=dst_ap, in_=xh[:])


def _build(tc, nc, xc, alibi, wq, wk, wv, bqkv, wo, w1, b1, w2,
           ffn_part, h_part, groups):
    import contextlib

    ctx = contextlib.ExitStack()
    with ctx:
        dram = ctx.enter_context(tc.tile_pool(name="dram", bufs=1, space="DRAM"))
        xh_in = [dram.tile([P, D], BF16, name=f"xh_in{u}") for u in range(NU)]
        xh_ag = [dram.tile([UTOK, D], BF16, addr_space="Shared",
                           name=f"xh_ag{u}") for u in range(NU)]
        att_pt = [dram.tile([UTOK, D], BF16, name=f"att_pt{u}")
                  for u in range(NU)]
        rs_out = [dram.tile([P, D], BF16, name=f"rs_out{u}") for u in range(NU)]
        hn_in = [dram.tile([P, D], BF16, name=f"hn_in{u}") for u in range(NU)]
        hn_ag = [dram.tile([UTOK, D], BF16, addr_space="Shared",
                           name=f"hn_ag{u}") for u in range(NU)]

        # one shared PSUM pool for the whole kernel (8 banks):
        # "big" rotating slots for every short-lived accumulation;
        # "avps"/"denps" live across an attention j-loop.
        psum = ctx.enter_context(tc.tile_pool(name="psum", bufs=1, space="PSUM"))

        # small constants
        wsm = ctx.enter_context(tc.tile_pool(name="wsm", bufs=1))
        eps_t = wsm.tile([P, 1], F32)
        nc.vector.memset(eps_t[:], LN_EPS)
        bqkv_sb = wsm.tile([P, 3 * HPC], F32)
        nc.gpsimd.dma_start(out=bqkv_sb[:],
                            in_=bqkv.rearrange("(a p) -> p a", p=P))
        bv_bc = wsm.tile([P, HPC * HD], F32)
        _bv = bqkv[2 * HPC * HD:3 * HPC * HD]
        nc.gpsimd.dma_start(
            out=bv_bc[:],
            in_=bass.AP(tensor=_bv.tensor, offset=_bv.offset,
                        ap=[[0, P]] + [list(a) for a in _bv.ap]))
        ones_bf = wsm.tile([P, 1], BF16)
        nc.vector.memset(ones_bf[:], 1.0)
        ones_row = wsm.tile([1, P], F32)
        nc.vector.memset(ones_row[:], 1.0)

        # attention-lifetime sbuf: qT/kT per head, v natural, wo
        attq = ctx.enter_context(tc.tile_pool(name="attq", bufs=1))
        qT = [attq.tile([P, NTOK], BF16, name=f"qT{h}") for h in range(HPC)]
        kT = [attq.tile([P, NTOK], BF16, name=f"kT{h}") for h in range(HPC)]
        vnat = [attq.tile([P, NQB, HPC * HD], BF16, name=f"vnat{b}")
                for b in range(B)]
        wo_sb = attq.tile([P, HPC, D], BF16)
        nc.gpsimd.dma_start(out=wo_sb[:], in_=wo.rearrange("(h p) o -> p h o", p=P))

        # ---------------- phase A+B: LN1 chunks + AllGather + QKV
        with tc.tile_pool(name="ln1", bufs=2) as ln1p, \
             tc.tile_pool(name="qkvw", bufs=1) as qwp, \
             tc.tile_pool(name="qkv", bufs=2) as qkvp:
            wq_sb = qwp.tile([P, KCH, HPC * HD], BF16)
            wk_sb = qwp.tile([P, KCH, HPC * HD], BF16)
            wv_sb = qwp.tile([P, KCH, HPC * HD], BF16)
            nc.gpsimd.dma_start(out=wq_sb[:],
                                in_=wq.rearrange("(c p) o -> p c o", p=P))
            nc.gpsimd.dma_start(out=wk_sb[:],
                                in_=wk.rearrange("(c p) o -> p c o", p=P))
            nc.gpsimd.dma_start(out=wv_sb[:],
                                in_=wv.rearrange("(c p) o -> p c o", p=P))
            for u in range(NU):
                _ln_tile(nc, ln1p, eps_t,
                         src_ap=xc[u * P:(u + 1) * P, :],
                         dst_ap=xh_in[u][:], tag=f"a{u}")
                nc.gpsimd.collective_compute(
                    "AllGather", mybir.AluOpType.bypass, replica_groups=groups,
                    ins=[xh_in[u].opt()], outs=[xh_ag[u].opt()])
            for u in range(NU):
                for half in range(UTOK // SPAN):
                    tok0 = u * UTOK + half * SPAN
                    xT = qkvp.tile([P, KCH, SPAN], BF16, tag="xT")
                    for kc in range(KCH):
                        nc.sync.dma_start(
                            out=xT[:, kc, :],
                            in_=xh_ag[u][half * SPAN:(half + 1) * SPAN,
                                         kc * P:(kc + 1) * P],
                            transpose=True)
                    for wi, (w_sb, outs) in enumerate(
                            ((wq_sb, qT), (wk_sb, kT))):
                        for h in range(HPC):
                            ps = psum.tile([P, SPAN], F32, tag="big", bufs=5,
                                           name=f"qk_{u}_{half}_{wi}_{h}")
                            for kc in range(KCH):
                                nc.tensor.matmul(
                                    ps[:],
                                    lhsT=w_sb[:, kc, h * HD:(h + 1) * HD],
                                    rhs=xT[:, kc, :],
                                    start=(kc == 0), stop=(kc == KCH - 1))
                            col = wi * HPC + h
                            nc.scalar.activation(
                                out=outs[h][:, tok0:tok0 + SPAN], in_=ps[:],
                                func=mybir.ActivationFunctionType.Identity,
                                bias=bqkv_sb[:, col:col + 1], scale=1.0)
                    # v natural: one [128-tok, 256] psum per token block
                    for tb in range(SPAN // P):
                        gtok = tok0 + tb * P
                        b, j = divmod(gtok, S)
                        j //= P
                        vp = psum.tile([P, HPC * HD], F32, tag="big", bufs=5,
                                       name=f"v_{u}_{half}_{tb}")
                        for kc in range(KCH):
                            nc.tensor.matmul(
                                vp[:],
                                lhsT=xT[:, kc, tb * P:(tb + 1) * P],
                                rhs=wv_sb[:, kc, :],
                                start=(kc == 0), stop=(kc == KCH - 1))
                        nc.vector.scalar_tensor_tensor(
                            out=vnat[b][:, j, :], in0=vp[:], scalar=1.0,
                            in1=bv_bc[:], op0=mybir.AluOpType.mult,
                            op1=mybir.AluOpType.add)

        # ---------------- phase C: attention + W_o partials (chunked RS)
        # Scores are computed TRANSPOSED (see module docstring).
        with tc.tile_pool(name="att", bufs=3) as ap_, \
             tc.tile_pool(name="att_sm", bufs=4) as smp, \
             tc.tile_pool(name="avt", bufs=8) as avtp, \
             tc.tile_pool(name="ln2", bufs=2) as ln2p:
            for b in range(B):
                for m in range(NSP):
                    nkb = 4 * (m + 1)        # causal: k blocks 0..4m+3
                    toff = b * S
                    avT = [None] * HPC
                    for h in range(HPC):
                        av_ps = psum.tile([P, SPAN], F32, tag="avps", bufs=2,
                                          name=f"avp_{b}_{m}_{h}")
                        den_ps = psum.tile([1, SPAN], F32, tag="denps", bufs=1,
                                           name=f"den_{b}_{m}_{h}")
                        for j in range(nkb):
                            qlo = max(0, j * P - m * SPAN)
                            nq = SPAN - qlo
                            al_t = ap_.tile([P, SPAN], F32, tag="alibi",
                                            name=f"al_{b}_{m}_{h}_{j}")
                            nc.gpsimd.dma_start(
                                out=al_t[:, 0:nq],
                                in_=alibi[h, j * P:(j + 1) * P,
                                          m * SPAN + qlo:(m + 1) * SPAN])
                            ps = psum.tile([P, SPAN], F32, tag="big", bufs=5,
                                           name=f"sc_{b}_{m}_{h}_{j}")
                            nc.tensor.matmul(
                                ps[:, 0:nq],
                                lhsT=kT[h][:, toff + j * P:toff + (j + 1) * P],
                                rhs=qT[h][:, toff + m * SPAN + qlo:
                                          toff + (m + 1) * SPAN],
                                start=True, stop=True)
                            s_t = smp.tile([P, SPAN], F32, tag="s",
                                           name=f"s_{b}_{m}_{h}_{j}")
                            nc.vector.scalar_tensor_tensor(
                                out=s_t[:, 0:nq], in0=ps[:, 0:nq], scalar=1.0,
                                in1=al_t[:, 0:nq], op0=mybir.AluOpType.mult,
                                op1=mybir.AluOpType.add)
                            ET_t = smp.tile([P, SPAN], BF16, tag="ET",
                                            name=f"ET_{b}_{m}_{h}_{j}")
                            nc.scalar.activation(
                                out=ET_t[:, 0:nq], in_=s_t[:, 0:nq],
                                func=mybir.ActivationFunctionType.Exp)
                            nc.tensor.matmul(
                                av_ps[:, qlo:],
                                lhsT=vnat[b][:, j, h * HD:(h + 1) * HD],
                                rhs=ET_t[:, 0:nq],
                                start=(j == 0), stop=(j == nkb - 1),
                                skip_group_check=True)
                            nc.tensor.matmul(
                                den_ps[:, qlo:], lhsT=ones_bf[:],
                                rhs=ET_t[:, 0:nq],
                                start=(j == 0), stop=(j == nkb - 1),
                                skip_group_check=True)
                        rec = smp.tile([1, SPAN], F32, tag="rec",
                                       name=f"rec_{b}_{m}_{h}")
                        nc.vector.reciprocal(out=rec[:], in_=den_ps[:])
                        bc_ps = psum.tile([P, SPAN], F32, tag="big", bufs=5,
                                          name=f"bcps_{b}_{m}_{h}")
                        nc.tensor.matmul(bc_ps[:], lhsT=ones_row[:],
                                         rhs=rec[:], start=True, stop=True)
                        rec_bc = smp.tile([P, SPAN], F32, tag="recbc",
                                          name=f"recbc_{b}_{m}_{h}")
                        nc.scalar.copy(out=rec_bc[:], in_=bc_ps[:])
                        avT_t = avtp.tile([P, SPAN], BF16, tag="avT",
                                          name=f"avT_{b}_{m}_{h}")
                        nc.vector.scalar_tensor_tensor(
                            out=avT_t[:], in0=av_ps[:], scalar=1.0,
                            in1=rec_bc[:], op0=mybir.AluOpType.mult,
                            op1=mybir.AluOpType.mult)
                        avT[h] = avT_t
                    # W_o for this q span (accumulate over local heads)
                    u, uoff = divmod(b * S + m * SPAN, UTOK)
                    for qb in range(SPAN // P):
                        for dsp in range(D // SPAN):
                            ps = psum.tile([P, SPAN], F32, tag="big", bufs=5,
                                           name=f"wo_{b}_{m}_{qb}_{dsp}")
                            for h in range(HPC):
                                nc.tensor.matmul(
                                    ps[:],
                                    lhsT=avT[h][:, qb * P:(qb + 1) * P],
                                    rhs=wo_sb[:, h,
                                              dsp * SPAN:(dsp + 1) * SPAN],
                                    start=(h == 0), stop=(h == HPC - 1))
                            o_sb = smp.tile([P, SPAN], BF16, tag="wo_o",
                                            name=f"woo_{b}_{m}_{qb}_{dsp}")
                            nc.any.tensor_copy(out=o_sb[:], in_=ps[:])
                            row = uoff + qb * P
                            nc.gpsimd.dma_start(
                                out=att_pt[u][row:row + P,
                                              dsp * SPAN:(dsp + 1) * SPAN],
                                in_=o_sb[:])
                    if m % 2 == 1:
                        # chunk u complete on every core -> RS, LN2, AG
                        nc.gpsimd.collective_compute(
                            "ReduceScatter", mybir.AluOpType.add,
                            replica_groups=groups,
                            ins=[att_pt[u].opt()], outs=[rs_out[u].opt()])
                        _ln_tile(nc, ln2p, eps_t,
                                 src_ap=xc[u * P:(u + 1) * P, :],
                                 dst_ap=hn_in[u][:], tag=f"b{u}",
                                 rs_ap=rs_out[u][:],
                                 h_ap=h_part[u * P:(u + 1) * P, :])
                        nc.gpsimd.collective_compute(
                            "AllGather", mybir.AluOpType.bypass,
                            replica_groups=groups,
                            ins=[hn_in[u].opt()], outs=[hn_ag[u].opt()])

        # ---------------- phase E: FFN (consumes hn_ag chunks)
        with tc.tile_pool(name="ffnw", bufs=1) as fwp, \
             tc.tile_pool(name="ffn", bufs=2) as ffnp:
            w1_sb = fwp.tile([P, KCH, DFFC], BF16)
            nc.gpsimd.dma_start(out=w1_sb[:],
                                in_=w1.rearrange("(c p) f -> p c f", p=P))
            b1_sb = fwp.tile([P, FSL], F32)
            nc.gpsimd.dma_start(out=b1_sb[:],
                                in_=b1.rearrange("(s p) -> p s", p=P))
            w2_sb = fwp.tile([P, FSL, D], BF16)
            nc.gpsimd.dma_start(out=w2_sb[:],
                                in_=w2.rearrange("(c p) o -> p c o", p=P))
            for u in range(NU):
                for half in range(UTOK // SPAN):
                    hsl = slice(half * SPAN, (half + 1) * SPAN)
                    hT = ffnp.tile([P, KCH, SPAN], BF16, tag="hT")
                    for kc in range(KCH):
                        nc.sync.dma_start(
                            out=hT[:, kc, :],
                            in_=hn_ag[u][hsl, kc * P:(kc + 1) * P],
                            transpose=True)
                    g1 = ffnp.tile([P, FSL, SPAN], BF16, tag="g1")
                    for s in range(FSL):
                        ps = psum.tile([P, SPAN], F32, tag="big", bufs=5,
                                       name=f"f1_{u}_{half}_{s}")
                        for kc in range(KCH):
                            nc.tensor.matmul(
                                ps[:], lhsT=w1_sb[:, kc, s * P:(s + 1) * P],
                                rhs=hT[:, kc, :],
                                start=(kc == 0), stop=(kc == KCH - 1))
                        nc.scalar.activation(
                            out=g1[:, s, :], in_=ps[:],
                            func=mybir.ActivationFunctionType.Gelu,
                            bias=b1_sb[:, s:s + 1], scale=1.0)
                    for tb in range(SPAN // P):
                        for dsp in range(D // SPAN):
                            ps2 = psum.tile([P, SPAN], F32, tag="big", bufs=5,
                                            name=f"f2_{u}_{half}_{tb}_{dsp}")
                            for s in range(FSL):
                                nc.tensor.matmul(
                                    ps2[:],
                                    lhsT=g1[:, s, tb * P:(tb + 1) * P],
                                    rhs=w2_sb[:, s,
                                              dsp * SPAN:(dsp + 1) * SPAN],
                                    start=(s == 0), stop=(s == FSL - 1))
                            o_sb = ffnp.tile([P, SPAN], BF16, tag="fo")
                            nc.any.tensor_copy(out=o_sb[:], in_=ps2[:])
                            row = u * UTOK + half * SPAN + tb * P
                            nc.gpsimd.dma_start(
                                out=ffn_part[row:row + P,
                                             dsp * SPAN:(dsp + 1) * SPAN],
                                in_=o_sb[:])


# ---------------------------------------------------------------- host side
def _prep_inputs(x, alibi_bias, W_q, W_k, W_v, W_o, ln1_g, ln1_b, ln2_g,
                 ln2_b, ffn_w1, ffn_b1, ffn_w2, ffn_b2):
    f32 = np.float32
    x = np.ascontiguousarray(np.asarray(x, f32).reshape(NTOK, D))
    inv_sqrt_hd = f32(1.0 / math.sqrt(HD))
    ln1_g = np.asarray(ln1_g, f32)
    ln1_b = np.asarray(ln1_b, f32)
    ln2_g = np.asarray(ln2_g, f32)
    ln2_b = np.asarray(ln2_b, f32)

    wq_f = (ln1_g[:, None] * np.asarray(W_q, f32)) * inv_sqrt_hd
    bq = (ln1_b @ np.asarray(W_q, f32)) * inv_sqrt_hd
    wk_f = ln1_g[:, None] * np.asarray(W_k, f32)
    bk = ln1_b @ np.asarray(W_k, f32)
    wv_f = ln1_g[:, None] * np.asarray(W_v, f32)
    bv = ln1_b @ np.asarray(W_v, f32)
    w1_f = ln2_g[:, None] * np.asarray(ffn_w1, f32)
    b1_f = ln2_b @ np.asarray(ffn_w1, f32) + np.asarray(ffn_b1, f32)

    # alibi with causal mask folded in, TRANSPOSED to [head, k, q]
    al = np.asarray(alibi_bias, f32).copy()
    iu = np.triu_indices(S, k=1)
    al[:, iu[0], iu[1]] = NEG
    al = np.ascontiguousarray(al.transpose(0, 2, 1))

    W_o = np.asarray(W_o, f32)
    w2 = np.asarray(ffn_w2, f32)

    in_maps = []
    for c in range(NCORES):
        hs = slice(c * HPC * HD, (c + 1) * HPC * HD)     # head-dim slice
        fs = slice(c * DFFC, (c + 1) * DFFC)             # ffn slice
        bqkv_c = np.concatenate([bq[hs], bk[hs], bv[hs]]).astype(f32)
        # owned tokens: blocks u*1024 + c*128, u = 0..3
        xc_rows = np.concatenate(
            [x[u * UTOK + c * P:u * UTOK + (c + 1) * P] for u in range(NU)])
        in_maps.append({
            "xc": np.ascontiguousarray(xc_rows),
            "alibi": np.ascontiguousarray(al[c * HPC:(c + 1) * HPC]),
            "wq": np.ascontiguousarray(wq_f[:, hs].astype(np_bf16)),
            "wk": np.ascontiguousarray(wk_f[:, hs].astype(np_bf16)),
            "wv": np.ascontiguousarray(wv_f[:, hs].astype(np_bf16)),
            "bqkv": bqkv_c,
            "wo": np.ascontiguousarray(W_o[hs, :].astype(np_bf16)),
            "w1": np.ascontiguousarray(w1_f[:, fs].astype(np_bf16)),
            "b1": np.ascontiguousarray(b1_f[fs]),
            "w2": np.ascontiguousarray(w2[fs, :].astype(np_bf16)),
        })
    return in_maps


def kernel(x, alibi_bias, W_q, W_k, W_v, W_o, ln1_g, ln1_b, ln2_g, ln2_b,
           ffn_w1, ffn_b1, ffn_w2, ffn_b2, *, _trace=False, _tmpdir=None):
    global LAST_RESULT
    if "nc" not in _CACHE:
        _CACHE["nc"] = build_program()
    nc = _CACHE["nc"]

    in_maps = _prep_inputs(x, alibi_bias, W_q, W_k, W_v, W_o, ln1_g, ln1_b,
                           ln2_g, ln2_b, ffn_w1, ffn_b1, ffn_w2, ffn_b2)

    res = run_bass_kernel_spmd(
        nc, in_maps, core_ids=list(range(NCORES)),
        trace=_trace, tmpdir=_tmpdir)
    LAST_RESULT = res

    out = np.zeros((NTOK, D), np.float32)
    for c in range(NCORES):
        out += np.asarray(res.results[c]["ffn_part"], np.float32)
    for c in range(NCORES):
        hp = np.asarray(res.results[c]["h_part"])
        for u in range(NU):
            out[u * UTOK + c * P:u * UTOK + (c + 1) * P] += \
                hp[u * P:(u + 1) * P]
    out += np.asarray(ffn_b2, np.float32)[None, :]
    return out.reshape(B, S, D)
